# revision 1
# baseline (speedup 1.0000x reference)
"""Trainium2 Bass kernel for causal multi-head attention (nn_Attention_5334349381821).

Problem: b=2, n=2048, dim=1024, 16 heads x 64 dim_head, RMSNorm + QKV + RoPE
(interleaved) + causal softmax attention + output projection.

Sharding: 8 cores = data-parallel on batch (2) x tensor-parallel on heads (4
groups of 4 heads). Each core computes a partial output [2048, 1024] through
its wo column-slice; host sums the 4 partials per batch element.

Layout strategy (per core):
  - xT = tokens[b].T  [1024(d), 2048(i)]  (host-transposed; d on partitions)
  - q/k computed transposed: qT [256(m), 2048(i)] via lhsT=wT_eff, rhs=xT
  - v computed natural:      v  [2048(j), 256(e)] via lhsT=xT, rhs=wvT_eff
  - S computed transposed:   S.T[j, i] via lhsT=kT (K=64), rhs=qT; two heads
    row-packed in the PE array (rows 0-63 / 64-127)
  - softmax denominator via ones-augmented v (M=65 AV matmuls)
  - RoPE: interleaved pair swap via DVE stream_shuffle; RMSNorm scale s[i] and
    rotation tables folded into CsT/SsT [128, 2048]
  - output projection back to natural [i, d] via lhsT=OT, rhs=woT
All matmuls run as float32r (fp32 data, full-rate PE mode).
"""

from contextlib import ExitStack

import numpy as np

import concourse.bass as bass
import concourse.tile as tile
from concourse import bacc, mybir
from concourse.bass_utils import run_bass_kernel_spmd

# Problem constants (hardcoded; kernel.py must be self-contained)
B = 2
N = 2048
DIM = 1024
HEADS = 16
DH = 64
N_CORES = 8
HEADS_PER_CORE = HEADS // (N_CORES // B)  # 4
M = HEADS_PER_CORE * DH  # 256 = per-core q/k/v width
RMS_EPS = 1.1920929e-07
ROPE_THETA = 10000.0

P = 128
F32 = mybir.dt.float32
F32R = mybir.dt.float32r

KT = DIM // P        # 8 k-tiles over dim
IT = N // P          # 16 token tiles of 128
NC = N // 512        # 4 chunks of 512 tokens
MT = M // P          # 2 m-tiles (= head-pairs)

SHUF_SWAP = [i ^ 1 for i in range(32)]
DEBUG_DUMPS = False
REPEATS = 1  # emit the body multiple times (for repeat-slope HW timing)


def build_program():
    nc = bacc.Bacc(
        "TRN2",
        target_bir_lowering=False,
        debug=False,
        enable_asserts=False,
        num_devices=N_CORES,
    )

    xT_d = nc.dram_tensor("xT", [DIM, N], F32R, kind="ExternalInput").ap()
    nw_d = nc.dram_tensor("nw_pt", [P, KT], F32R, kind="ExternalInput").ap()
    wqT_d = nc.dram_tensor("wqT", [DIM, M], F32R, kind="ExternalInput").ap()
    wkT_d = nc.dram_tensor("wkT", [DIM, M], F32R, kind="ExternalInput").ap()
    wvT_d = nc.dram_tensor("wvT", [DIM, M], F32R, kind="ExternalInput").ap()
    woT_d = nc.dram_tensor("woT", [M, DIM], F32R, kind="ExternalInput").ap()
    cos_d = nc.dram_tensor("cos_t", [P, N], F32, kind="ExternalInput").ap()
    sin_d = nc.dram_tensor("sin_t", [P, N], F32, kind="ExternalInput").ap()
    tri_d = nc.dram_tensor("tri", [P, P], F32R, kind="ExternalInput").ap()
    out_d = nc.dram_tensor("out_part", [N, DIM], F32, kind="ExternalOutput").ap()
    dbg = {}
    if DEBUG_DUMPS:
        for nm, shape in [("d_srow", [1, N]), ("d_cs", [P, N]), ("d_ss", [P, N]),
                          ("d_qT0", [P, N]), ("d_kT0", [P, N]), ("d_vaug", [P, IT * HEADS_PER_CORE * (DH + 1)]),
                          ("d_OT0", [P, N]), ("d_stcol", [P, IT])]:
            dbg[nm] = nc.dram_tensor(nm, shape, F32, kind="ExternalOutput").ap()

    with tile.TileContext(nc) as tc:
        for _rep in range(REPEATS):
            _emit(nc, tc, xT_d, nw_d, wqT_d, wkT_d, wvT_d, woT_d, cos_d, sin_d, tri_d, out_d, dbg)

    nc.compile()
    return nc


def _emit(nc, tc, xT_d, nw_d, wqT_d, wkT_d, wvT_d, woT_d, cos_d, sin_d, tri_d, out_d, dbg={}):
    MULT = mybir.AluOpType.mult
    ADD = mybir.AluOpType.add
    EXPF = mybir.ActivationFunctionType.Exp

    with ExitStack() as whole:
        # ---------- long-lived pools ----------
        persist = whole.enter_context(tc.tile_pool(name="persist", bufs=1))

        # packed constants: tri [0:128], sT_col [128:144], nw [144:152], ones [152]
        consts = persist.tile([P, 256], F32R, name="consts", tag="consts")
        tri = consts[:, 0:128]
        sT_col = consts[:, 128:144]
        nw = consts[:, 144:152]
        ones_col = consts[:, 152:153]
        nc.sync.dma_start(tri, tri_d[:])
        nc.sync.dma_start(nw, nw_d.rearrange("p o -> p o"))
        nc.vector.memset(ones_col.bitcast(F32), 1.0)

        wo = persist.tile([P, MT, DIM], F32R, name="wo", tag="wo")
        nc.sync.dma_start(wo[:], woT_d.rearrange("(o p) d -> p o d", p=P))

        qT = [persist.tile([P, N], F32R, name=f"qT{mt}", tag=f"qT{mt}") for mt in range(MT)]
        kTt = [persist.tile([P, N], F32R, name=f"kT{mt}", tag=f"kT{mt}") for mt in range(MT)]
        v_aug = persist.tile([P, IT, HEADS_PER_CORE, DH + 1], F32R, name="v_aug", tag="v_aug")
        OT = [persist.tile([P, N], F32R, name=f"OT{mt}", tag=f"OT{mt}") for mt in range(MT)]

        with ExitStack() as xphase:
            # ---------- loads ----------
            xpool = xphase.enter_context(tc.tile_pool(name="xpool", bufs=1))
            xT = []
            for kt in range(KT):
                t = xpool.tile([P, N], F32R, name=f"xT{kt}", tag=f"xT{kt}")
                for half in range(2):
                    hs = slice(half * (N // 2), (half + 1) * (N // 2))
                    nc.sync.dma_start(t[:, hs], xT_d[kt * P : (kt + 1) * P, hs])
                xT.append(t)
            cos_t = xpool.tile([P, N], F32, name="cos_t", tag="cos")
            nc.sync.dma_start(cos_t[:], cos_d[:])
            sin_t = xpool.tile([P, N], F32, name="sin_t", tag="sin")
            nc.sync.dma_start(sin_t[:], sin_d[:])

            # ---------- phase 1: RMSNorm scale ----------
            with tc.tile_pool(name="ph1", bufs=2) as ph1, \
                 tc.tile_pool(name="ps_ssq", bufs=1, space="PSUM") as ps_ssq:
                s_row = ph1.tile([1, N], F32, name="s_row", tag="s_row")
                eps_t = ph1.tile([1, 1], F32, name="eps_t", tag="eps_t")
                nc.vector.memset(eps_t[:], RMS_EPS)
                s_bcast = ph1.tile([P, N], F32, name="s_bcast", tag="s_bcast")
                ssq_ps = [
                    ps_ssq.tile([1, 512], F32, name=f"ssq{c}", tag=f"ssq{c}")
                    for c in range(NC)
                ]
                for kt in range(KT):
                    sq = ph1.tile([P, N], F32R, name="sq", tag="sq")
                    for half in range(2):
                        hh = slice(half * (N // 2), (half + 1) * (N // 2))
                        nc.scalar.square(sq[:, hh], xT[kt][:, hh])
                    for c in range(NC):
                        cs = slice(c * 512, (c + 1) * 512)
                        nc.tensor.matmul(
                            ssq_ps[c][:], (ones_col), (sq[:, cs]),
                            start=(kt == 0), stop=(kt == KT - 1),
                        )
                for c in range(NC):
                    cs = slice(c * 512, (c + 1) * 512)
                    rt = ph1.tile([1, 512], F32, name="rt", tag="rt")
                    nc.scalar.activation(
                        rt[:], ssq_ps[c][:], mybir.ActivationFunctionType.Sqrt,
                        bias=eps_t[:], scale=1.0 / DIM,
                    )
                    nc.vector.reciprocal(s_row[:, cs], rt[:])

                nc.gpsimd.partition_broadcast(s_bcast[:], s_row[:])
                nc.vector.tensor_tensor(cos_t[:], cos_t[:], s_bcast[:], MULT)
                nc.vector.tensor_tensor(sin_t[:], sin_t[:], s_bcast[:], MULT)
                # s in token-partition layout: SBUF->SBUF partition-crossing DMA
                # corrupts on HW, so bounce via DRAM
                with tc.tile_pool(name="dram_s", bufs=1, space="DRAM") as dram_pool:
                    s_dram = dram_pool.tile([1, N], F32, name="s_dram", tag="s_dram")
                    nc.sync.dma_start(s_dram[:], s_row[:])
                    nc.sync.dma_start(
                        sT_col,
                        s_dram[0, :].rearrange("(t p) -> p t", p=P).bitcast(F32R),
                    )
                if dbg:
                    nc.sync.dma_start(dbg["d_srow"], s_row[:])
                    nc.sync.dma_start(dbg["d_cs"], cos_t[:])
                    nc.sync.dma_start(dbg["d_ss"], sin_t[:])

            # ---------- phase 2/3: projections ----------
            with tc.tile_pool(name="wpool", bufs=3) as wpool, \
                 tc.tile_pool(name="rope", bufs=4) as rope, \
                 tc.tile_pool(name="ps_prj", bufs=6, space="PSUM") as ps_prj, \
                 tc.tile_pool(name="ps_v", bufs=2, space="PSUM") as ps_v:
                wv = wpool.tile([P, KT, M], F32R, name="wv", tag="w")
                nc.sync.dma_start(wv[:], wvT_d.rearrange("(o p) m -> p o m", p=P))
                nc.vector.tensor_tensor(
                    wv[:], wv[:], nw[:, :, None].to_broadcast([P, KT, M]), MULT
                )
                nc.vector.memset(v_aug[:, :, :, DH : DH + 1].bitcast(F32), 1.0)
                for jt in range(IT):
                    vp = ps_v.tile([P, M], F32, name=f"v_ps{jt}", tag="v_ps")
                    for kt in range(KT):
                        nc.tensor.matmul(
                            vp[:],
                            (xT[kt][:, jt * P : (jt + 1) * P]),
                            (wv[:, kt, :]),
                            start=(kt == 0), stop=(kt == KT - 1),
                        )
                    nc.vector.tensor_scalar_mul(
                        v_aug[:, jt, :, 0:DH],
                        vp.rearrange("p (h e) -> p h e", h=HEADS_PER_CORE),
                        sT_col[:, jt : jt + 1].bitcast(F32),
                    )

                wq_sb = wpool.tile([P, KT, M], F32R, name="wq_sb", tag="w")
                nc.sync.dma_start(wq_sb[:], wqT_d.rearrange("(o p) m -> p o m", p=P))
                nc.vector.tensor_tensor(
                    wq_sb[:], wq_sb[:], nw[:, :, None].to_broadcast([P, KT, M]), MULT
                )
                wk_sb = wpool.tile([P, KT, M], F32R, name="wk_sb", tag="w")
                nc.sync.dma_start(wk_sb[:], wkT_d.rearrange("(o p) m -> p o m", p=P))
                nc.vector.tensor_tensor(
                    wk_sb[:], wk_sb[:], nw[:, :, None].to_broadcast([P, KT, M]), MULT
                )
                for mt in range(MT):
                    for w, dst in ((wq_sb, qT), (wk_sb, kTt)):
                        acc = [
                            ps_prj.tile([P, 512], F32, name=f"prj{mt}_{c}", tag="prj")
                            for c in range(NC)
                        ]
                        for kt in range(KT):
                            lhs = w[:, kt, mt * P : (mt + 1) * P]
                            for c in range(NC):
                                nc.tensor.matmul(
                                    acc[c][:], (lhs),
                                    (xT[kt][:, c * 512 : (c + 1) * 512]),
                                    start=(kt == 0), stop=(kt == KT - 1),
                                )
                        for c in range(NC):
                            cs = slice(c * 512, (c + 1) * 512)
                            sw = rope.tile([P, 512], F32, name="sw", tag="sw")
                            nc.vector.stream_shuffle(sw[:], acc[c][:], SHUF_SWAP)
                            t1 = rope.tile([P, 512], F32, name="t1", tag="t1")
                            nc.vector.tensor_tensor(t1[:], acc[c][:], cos_t[:, cs], MULT)
                            nc.vector.tensor_tensor(sw[:], sw[:], sin_t[:, cs], MULT)
                            nc.gpsimd.tensor_tensor(dst[mt][:, cs], t1[:], sw[:], ADD)

        # ---------- phase 4: attention (c-outer) + interleaved output proj ----------
        with tc.tile_pool(name="ppool", bufs=6) as ppool, \
             tc.tile_pool(name="dpool", bufs=2) as dpool, \
             tc.tile_pool(name="ps_s", bufs=3, space="PSUM") as ps_s, \
             tc.tile_pool(name="ps_o", bufs=2, space="PSUM") as ps_o, \
             tc.tile_pool(name="ps_out", bufs=1, space="PSUM") as ps_out, \
             tc.tile_pool(name="opool", bufs=3) as opool:
            for c in range(NC):
                for hp in range(MT):
                    pO = [
                        ps_o.tile([DH + 1, 512], F32, name=f"o{h}_{hp}_{c}", tag=f"pO{h}")
                        for h in range(2)
                    ]
                    tmax = 4 * (c + 1)
                    for t in range(tmax):
                        off = max(0, t - 4 * c) * P
                        nt = 512 - off
                        i_lo = c * 512 + off
                        pS = []
                        for h in range(2):
                            base = h * DH
                            ps_t = ps_s.tile([P, 512], F32, name=f"s{h}", tag="sS")
                            nc.tensor.matmul(
                                ps_t[:, 0:nt],
                                (kTt[hp][base : base + DH, t * P : (t + 1) * P]),
                                (qT[hp][base : base + DH, i_lo : (c + 1) * 512]),
                                start=True, stop=True,
                                tile_position=(base, 0),
                            )
                            pS.append(ps_t)
                        pt = ppool.tile([P, 2, 512], F32R, name="pt", tag="pt")
                        for h in range(2):
                            nc.scalar.activation(
                                pt[:, h, 0:nt], pS[h][:, 0:nt], EXPF, scale=DH ** -0.5
                            )
                        if t >= 4 * c:
                            nc.vector.tensor_tensor(
                                pt[:, :, 0:P], pt[:, :, 0:P],
                                tri[:, None, :].to_broadcast([P, 2, P]), MULT,
                            )
                        for h in range(2):
                            nc.tensor.matmul(
                                pO[h][:, off : off + nt],
                                (v_aug[:, t, hp * 2 + h, :]),
                                (pt[:, h, 0:nt]),
                                start=(t == 0), stop=(t == tmax - 1),
                                skip_group_check=True,
                            )
                    for h in range(2):
                        den = dpool.tile([1, 512], F32, name="den", tag="den")
                        nc.vector.reciprocal(den[:], pO[h][DH : DH + 1, :])
                        recb = dpool.tile([DH, 512], F32, name="recb", tag="recb")
                        nc.gpsimd.partition_broadcast(recb[:], den[:])
                        nc.vector.tensor_tensor(
                            OT[hp][h * DH : (h + 1) * DH, c * 512 : (c + 1) * 512],
                            pO[h][0:DH, :], recb[:], MULT,
                        )
                # output projection for this i-chunk (both head-pairs done)
                for it in range(c * 4, (c + 1) * 4):
                    for dc in range(DIM // 512):
                        po = ps_out.tile([P, 512], F32, name=f"out{it}_{dc}", tag="out_ps")
                        for et in range(MT):
                            nc.tensor.matmul(
                                po[:],
                                (OT[et][:, it * P : (it + 1) * P]),
                                (wo[:, et, dc * 512 : (dc + 1) * 512]),
                                start=(et == 0), stop=(et == MT - 1),
                            )
                        osb = opool.tile([P, 512], F32, name="osb", tag="osb")
                        nc.vector.tensor_copy(osb[:], po[:])
                        nc.sync.dma_start(
                            out_d[it * P : (it + 1) * P, dc * 512 : (dc + 1) * 512],
                            osb[:],
                        )
            if dbg:
                nc.sync.dma_start(dbg["d_qT0"], qT[0].bitcast(F32)[:])
                nc.sync.dma_start(dbg["d_kT0"], kTt[0].bitcast(F32)[:])
                nc.sync.dma_start(dbg["d_vaug"], v_aug.bitcast(F32).rearrange("p a b c -> p (a b c)"))
                nc.sync.dma_start(dbg["d_stcol"], sT_col.bitcast(F32))
                nc.sync.dma_start(dbg["d_OT0"], OT[0].bitcast(F32)[:])


def _rope_tables():
    inv_freq = 1.0 / (ROPE_THETA ** (np.arange(0, DH, 2, dtype=np.float64) / DH))
    t = np.arange(N, dtype=np.float64)
    freqs = t[:, None] * inv_freq[None, :]  # [N, 32]
    cos = np.cos(freqs)
    sin = np.sin(freqs)
    rows = np.arange(P)
    tidx = (rows % DH) // 2
    cos_t = cos[:, tidx].T.astype(np.float32)  # [128, N]
    sign = np.where(rows % 2 == 0, -1.0, 1.0)
    sin_t = (sin[:, tidx] * sign[None, :]).T.astype(np.float32)
    return np.ascontiguousarray(cos_t), np.ascontiguousarray(sin_t)


def shard_inputs(tokens, norm_weight, wq, wk, wv, wo):
    """Build the 8 per-core input dicts (pure numpy layout prep)."""
    tokens = np.asarray(tokens, dtype=np.float32)
    norm_weight = np.asarray(norm_weight, dtype=np.float32)
    wq, wk, wv, wo = (np.asarray(w, dtype=np.float32) for w in (wq, wk, wv, wo))

    cos_t, sin_t = _rope_tables()
    tri = np.triu(np.ones((P, P), dtype=np.float32))  # keep j <= i (row=j, col=i)
    nw_pt = np.ascontiguousarray(norm_weight.reshape(KT, P).T)

    in_maps = []
    for c in range(N_CORES):
        b = c // (N_CORES // B)
        g = c % (N_CORES // B)
        sl = slice(g * M, (g + 1) * M)
        in_maps.append({
            "xT": np.ascontiguousarray(tokens[b].T),
            "nw_pt": nw_pt,
            "wqT": np.ascontiguousarray(wq[sl, :].T),
            "wkT": np.ascontiguousarray(wk[sl, :].T),
            "wvT": np.ascontiguousarray(wv[sl, :].T),
            "woT": np.ascontiguousarray(wo[:, sl].T),
            "cos_t": cos_t,
            "sin_t": sin_t,
            "tri": tri,
        })
    return in_maps


_PROGRAM = None


def _get_program():
    global _PROGRAM
    if _PROGRAM is None:
        _PROGRAM = build_program()
    return _PROGRAM


def run(tokens, norm_weight, wq, wk, wv, wo, trace=False, **run_kwargs):
    nc = _get_program()
    in_maps = shard_inputs(tokens, norm_weight, wq, wk, wv, wo)
    res = run_bass_kernel_spmd(
        nc, in_maps, core_ids=list(range(N_CORES)), trace=trace, **run_kwargs
    )
    parts = [r["out_part"] for r in res.results]
    out = np.zeros((B, N, DIM), dtype=np.float64)
    for c in range(N_CORES):
        out[c // (N_CORES // B)] += parts[c].astype(np.float64)
    return out.astype(np.float32), res


def kernel(tokens, norm_weight, wq, wk, wv, wo):
    out, _ = run(tokens, norm_weight, wq, wk, wv, wo)
    return out

